# revision 6
# baseline (speedup 1.0000x reference)
"""PatchCore anomaly detection kernel for 8x Trainium2 NeuronCores.

Strategy:
  - Host (numpy): avg-pools, nearest-upsample, concat -> embedding [12544, 1536];
    row norms; gaussian blur; final softmax-weighted score.
  - Device (8 cores, memory bank sharded 2048 rows/core): for every patch i,
    q_c[i] = max_j (emb_i . bank_j - 0.5*||bank_j||^2) over the core's shard.
    This is the only O(N*M*D) work.  min-distance falls out as
    d2_i = ||emb_i||^2 - 2 * max_c q_c[i]  (min over full bank).
  - The top-9 row needed for the confidence weights involves one patch only
    (argmax of s0) and is recomputed exactly on host.
"""

import sys
import types

import numpy as np
import ml_dtypes

# bass_utils' axon trace path imports antenv.axon_hooks, which is absent in
# some containers; provide a graceful no-trace stub so a stray BASS_TRACE=1
# cannot crash the run.
try:
    import antenv.axon_hooks  # noqa: F401
except ImportError:
    try:
        import antenv
        _stub = types.ModuleType("antenv.axon_hooks")
        _stub.get_axon_ntff_profile_hook = lambda: None
        sys.modules["antenv.axon_hooks"] = _stub
        antenv.axon_hooks = _stub
    except ImportError:
        pass

B, H2, W2 = 16, 28, 28
D = 1536
NPATCH = B * H2 * W2          # 12544
MBANK = 16384
NCORES = 8
MS = MBANK // NCORES          # 2048 bank rows per core
KCH = D // 128                # 12 contraction chunks
NBLK = NPATCH // 128          # 98 patch blocks
JT = MS // 512                # 4 bank column tiles per core
SUPER = 2048                  # patches per streamed embedding super-block

SIGMA = 4.0
KS = 2 * int(4.0 * SIGMA + 0.5) + 1   # 33
INPUT_SIZE = 224
NUM_NEIGHBORS = 9

_NC_CACHE = None
LAST_RESULTS = None   # BassKernelResults of the most recent device run
LAST_IN_MAPS = None   # per-core input dicts of the most recent device run


def _build_nc(loop_k=None):
    """Build the device program.  loop_k: if set, wrap the whole body in an
    on-device For_i repeat loop (bench builds only)."""
    import contextlib
    import concourse.bacc as bacc
    import concourse.mybir as mybir
    import concourse.tile as tile

    nc = bacc.Bacc()
    embT = nc.dram_tensor("embT", [KCH, 128, NPATCH], mybir.dt.bfloat16,
                          kind="ExternalInput")
    bankT = nc.dram_tensor("bankT", [KCH, 128, MS], mybir.dt.bfloat16,
                           kind="ExternalInput")
    m2h = nc.dram_tensor("m2h", [MS], mybir.dt.float32, kind="ExternalInput")
    qmax = nc.dram_tensor("qmax", [128, NBLK], mybir.dt.float32,
                          kind="ExternalOutput")

    f32 = mybir.dt.float32
    bf16 = mybir.dt.bfloat16
    X = mybir.AxisListType.X
    SUB = mybir.AluOpType.subtract

    starts = list(range(0, NPATCH, SUPER))  # last super is 256 wide

    with tile.TileContext(nc) as tc:
        with (
            tc.tile_pool(name="singles", bufs=1) as singles,
            tc.tile_pool(name="embp", bufs=2) as embp,
            tc.tile_pool(name="scratch", bufs=4) as scratch,
            tc.tile_pool(name="res", bufs=2) as resp,
            tc.tile_pool(name="psum", bufs=8, space="PSUM") as psump,
        ):
            rep = tc.For_i(0, loop_k, 1) if loop_k else contextlib.nullcontext()
            with rep:
                bank_sb = singles.tile([128, KCH, MS], bf16)
                nc.sync.dma_start(bank_sb[:], bankT.rearrange("k p n -> p k n"))
                m2_sb = singles.tile([128, MS], f32)
                nc.sync.dma_start(m2_sb[:], m2h[None, :].to_broadcast((128, MS)))
                qout = singles.tile([128, NBLK], f32)

                for n0 in starts:
                    w = min(SUPER, NPATCH - n0)
                    emb_sb = embp.tile([128, KCH, SUPER], bf16, tag="emb")
                    nc.sync.dma_start(
                        emb_sb[:, :, :w],
                        embT[:, :, n0:n0 + w].rearrange("k p n -> p k n"),
                    )
                    for b in range(w // 128):
                        blk = n0 // 128 + b
                        res4 = resp.tile([128, JT], f32, tag="res4")
                        for j in range(JT):
                            ps = psump.tile([128, 512], f32, tag="ps")
                            for k in range(KCH):
                                nc.tensor.matmul(
                                    ps[:],
                                    emb_sb[:, k, b * 128:(b + 1) * 128],
                                    bank_sb[:, k, j * 512:(j + 1) * 512],
                                    start=(k == 0),
                                    stop=(k == KCH - 1),
                                )
                            tmp = scratch.tile([128, 512], f32, tag="tmp")
                            nc.vector.tensor_tensor(tmp[:], ps[:],
                                                    m2_sb[:, j * 512:(j + 1) * 512],
                                                    SUB)
                            nc.vector.reduce_max(res4[:, j:j + 1], tmp[:], axis=X)
                        nc.vector.reduce_max(qout[:, blk:blk + 1], res4[:], axis=X)

                nc.sync.dma_start(qmax[:], qout[:])

    nc.compile()
    return nc


def _get_nc():
    global _NC_CACHE
    if _NC_CACHE is None:
        _NC_CACHE = _build_nc()
    return _NC_CACHE


def _avg_pool3(x):
    b, c, h, w = x.shape
    xp = np.zeros((b, c, h + 2, w + 2), dtype=np.float32)
    xp[:, :, 1:h + 1, 1:w + 1] = x
    s = np.zeros((b, c, h, w), dtype=np.float32)
    for di in range(3):
        for dj in range(3):
            s += xp[:, :, di:di + h, dj:dj + w]
    return s / 9.0


def _blur_matrix():
    x = np.arange(KS, dtype=np.float64) - (KS - 1) / 2.0
    g = np.exp(-0.5 * (x / SIGMA) ** 2)
    g = (g / g.sum()).astype(np.float32)
    p = KS // 2
    M = np.zeros((INPUT_SIZE, INPUT_SIZE), dtype=np.float32)
    for o in range(INPUT_SIZE):
        for t in range(KS):
            idx = o + t - p
            if idx < 0:
                idx = -idx
            elif idx > INPUT_SIZE - 1:
                idx = 2 * (INPUT_SIZE - 1) - idx
            M[o, idx] += g[t]
    return M


def kernel(feat_layer2, feat_layer3, memory_bank):
    from concourse.bass_utils import run_bass_kernel_spmd
    global LAST_RESULTS, LAST_IN_MAPS

    feat_layer2 = np.asarray(feat_layer2, dtype=np.float32)
    feat_layer3 = np.asarray(feat_layer3, dtype=np.float32)
    memory_bank = np.asarray(memory_bank, dtype=np.float32)

    # ---- host: build the embedding ------------------------------------
    f2 = _avg_pool3(feat_layer2)                       # [16,512,28,28]
    f3 = _avg_pool3(feat_layer3)                       # [16,1024,14,14]
    f3u = f3.repeat(2, axis=2).repeat(2, axis=3)       # [16,1024,28,28]
    emb = np.concatenate([f2, f3u], axis=1)            # [16,1536,28,28]
    emb = np.ascontiguousarray(emb.transpose(0, 2, 3, 1)).reshape(NPATCH, D)

    embT = np.ascontiguousarray(emb.T)                 # [1536, 12544]
    embT_bf = embT.astype(ml_dtypes.bfloat16).reshape(KCH, 128, NPATCH)

    x2 = np.einsum("nd,nd->n", emb, emb).astype(np.float32)       # [12544]
    m2 = np.einsum("md,md->m", memory_bank, memory_bank).astype(np.float32)

    in_maps = []
    for c in range(NCORES):
        shard = memory_bank[c * MS:(c + 1) * MS]
        bankT_bf = np.ascontiguousarray(shard.T).astype(
            ml_dtypes.bfloat16).reshape(KCH, 128, MS)
        in_maps.append({
            "embT": embT_bf,
            "bankT": bankT_bf,
            "m2h": (0.5 * m2[c * MS:(c + 1) * MS]).astype(np.float32),
        })

    # ---- device: max_j (emb.bank_j - 0.5*m2_j) per patch per shard ----
    LAST_IN_MAPS = in_maps
    nc = _get_nc()
    LAST_RESULTS = run_bass_kernel_spmd(nc, in_maps, list(range(NCORES)))
    qs = np.stack([LAST_RESULTS.results[c]["qmax"].T.reshape(-1)
                   for c in range(NCORES)])            # [8, 12544]
    qglob = qs.max(axis=0)

    d2 = x2 - 2.0 * qglob
    s0 = np.sqrt(np.maximum(d2, 1e-12)).astype(np.float32)        # [12544]

    # ---- anomaly map: upsample x8 + separable gaussian blur -----------
    amap = s0.reshape(B, H2, W2).repeat(8, axis=1).repeat(8, axis=2)
    Mb = _blur_matrix()
    amap = np.matmul(np.matmul(Mb[None], amap), Mb.T[None])
    amap = amap.reshape(B, 1, INPUT_SIZE, INPUT_SIZE).astype(np.float32)

    # ---- anomaly score: exact top-9 for the argmax patch --------------
    ncand = 32
    cand = np.argpartition(-s0, ncand)[:ncand]
    ecand = emb[cand]                                   # [C, D]
    d2c = x2[cand][:, None] + m2[None, :] - 2.0 * (ecand @ memory_bank.T)
    dc = np.sqrt(np.maximum(d2c, 1e-12)).astype(np.float32)   # [C, 16384]
    s0c = dc.min(axis=1)
    ib = int(np.argmax(s0c))
    smax = np.float32(s0c[ib])
    drow = dc[ib]
    conf = np.sort(np.partition(drow, NUM_NEIGHBORS - 1)[:NUM_NEIGHBORS])
    econf = np.exp(conf)
    weights = np.float32(1.0 - econf.max() / econf.sum())
    anomaly_score = np.float32(weights * smax)

    return amap, anomaly_score


# revision 9
# speedup vs baseline: 2.0368x; 2.0368x over previous
"""PatchCore anomaly detection kernel for 8x Trainium2 NeuronCores.

Strategy:
  - Host (numpy): avg-pools, nearest-upsample, concat -> embedding [12544, 1536];
    row norms; gaussian blur; final softmax-weighted score.
  - Device (8 cores, memory bank sharded 2048 rows/core): for every patch i,
    q_c[i] = max_j (emb_i . bank_j - 0.5*||bank_j||^2) over the core's shard.
    This is the only O(N*M*D) work.  min-distance falls out as
    d2_i = ||emb_i||^2 - 2 * max_c q_c[i]  (min over full bank).
  - The top-9 row needed for the confidence weights involves one patch only
    (argmax of s0) and is recomputed exactly on host.
"""

import sys
import types

import numpy as np
import ml_dtypes

# bass_utils' axon trace path imports antenv.axon_hooks, which is absent in
# some containers; provide a graceful no-trace stub so a stray BASS_TRACE=1
# cannot crash the run.
try:
    import antenv.axon_hooks  # noqa: F401
except ImportError:
    try:
        import antenv
        _stub = types.ModuleType("antenv.axon_hooks")
        _stub.get_axon_ntff_profile_hook = lambda: None
        sys.modules["antenv.axon_hooks"] = _stub
        antenv.axon_hooks = _stub
    except ImportError:
        pass

B, H2, W2 = 16, 28, 28
D = 1536
NPATCH = B * H2 * W2          # 12544
MBANK = 16384
NCORES = 8
MS = MBANK // NCORES          # 2048 bank rows per core
KCH = D // 128                # 12 contraction chunks
NBLK = NPATCH // 128          # 98 patch blocks
JT = MS // 512                # 4 bank column tiles per core
SUPER = 2048                  # patches per streamed embedding super-block

SIGMA = 4.0
KS = 2 * int(4.0 * SIGMA + 0.5) + 1   # 33
INPUT_SIZE = 224
NUM_NEIGHBORS = 9

_NC_CACHE = None
LAST_RESULTS = None   # BassKernelResults of the most recent device run
LAST_IN_MAPS = None   # per-core input dicts of the most recent device run


USE_FP8 = True


def _build_nc(loop_k=None, use_fp8=None):
    """Build the device program.  loop_k: if set, wrap the whole body in an
    on-device For_i repeat loop (bench builds only)."""
    import contextlib
    import concourse.bacc as bacc
    import concourse.mybir as mybir
    import concourse.tile as tile

    if use_fp8 is None:
        use_fp8 = USE_FP8
    in_dt = mybir.dt.float8e4 if use_fp8 else mybir.dt.bfloat16
    kstep = 2 if use_fp8 else 1
    pm = mybir.MatmulPerfMode.DoubleRow if use_fp8 else None

    nc = bacc.Bacc()
    embT = nc.dram_tensor("embT", [KCH, 128, NPATCH], in_dt,
                          kind="ExternalInput")
    bankT = nc.dram_tensor("bankT", [KCH, 128, MS], in_dt,
                           kind="ExternalInput")
    m2h = nc.dram_tensor("m2h", [MS], mybir.dt.float32, kind="ExternalInput")
    qmax = nc.dram_tensor("qmax", [128, NBLK], mybir.dt.float32,
                          kind="ExternalOutput")

    f32 = mybir.dt.float32
    X = mybir.AxisListType.X
    SUB = mybir.AluOpType.subtract

    starts = list(range(0, NPATCH, SUPER))  # last super is 256 wide

    with tile.TileContext(nc) as tc:
        with (
            tc.tile_pool(name="singles", bufs=1) as singles,
            tc.tile_pool(name="embp", bufs=2) as embp,
            tc.tile_pool(name="scratch", bufs=4) as scratch,
            tc.tile_pool(name="res", bufs=2) as resp,
            tc.tile_pool(name="psum", bufs=8, space="PSUM") as psump,
        ):
            rep = tc.For_i(0, loop_k, 1) if loop_k else contextlib.nullcontext()
            with rep:
                bank_sb = singles.tile([128, KCH, MS], in_dt)
                nc.sync.dma_start(bank_sb[:], bankT.rearrange("k p n -> p k n"))
                m2_sb = singles.tile([128, MS], f32)
                nc.sync.dma_start(m2_sb[:], m2h[None, :].to_broadcast((128, MS)))
                qout = singles.tile([128, NBLK], f32)

                for n0 in starts:
                    w = min(SUPER, NPATCH - n0)
                    emb_sb = embp.tile([128, KCH, SUPER], in_dt, tag="emb")
                    nc.sync.dma_start(
                        emb_sb[:, :, :w],
                        embT[:, :, n0:n0 + w].rearrange("k p n -> p k n"),
                    )
                    for b in range(w // 128):
                        blk = n0 // 128 + b
                        res4 = resp.tile([128, JT], f32, tag="res4")
                        for j in range(JT):
                            ps = psump.tile([128, 512], f32, tag="ps")
                            for k in range(0, KCH, kstep):
                                lhsT = (emb_sb[:, k:k + 2, b * 128:(b + 1) * 128]
                                        if kstep == 2 else
                                        emb_sb[:, k, b * 128:(b + 1) * 128])
                                rhs = (bank_sb[:, k:k + 2, j * 512:(j + 1) * 512]
                                       if kstep == 2 else
                                       bank_sb[:, k, j * 512:(j + 1) * 512])
                                nc.tensor.matmul(
                                    ps[:], lhsT, rhs,
                                    start=(k == 0),
                                    stop=(k + kstep >= KCH),
                                    perf_mode=pm,
                                )
                            tmp = scratch.tile([128, 512], f32, tag="tmp")
                            nc.vector.tensor_tensor(tmp[:], ps[:],
                                                    m2_sb[:, j * 512:(j + 1) * 512],
                                                    SUB)
                            nc.vector.reduce_max(res4[:, j:j + 1], tmp[:], axis=X)
                        nc.vector.reduce_max(qout[:, blk:blk + 1], res4[:], axis=X)

                nc.sync.dma_start(qmax[:], qout[:])

    nc.compile()
    return nc


def _get_nc():
    global _NC_CACHE
    if _NC_CACHE is None:
        _NC_CACHE = _build_nc()
    return _NC_CACHE


def _avg_pool3(x):
    b, c, h, w = x.shape
    xp = np.zeros((b, c, h + 2, w + 2), dtype=np.float32)
    xp[:, :, 1:h + 1, 1:w + 1] = x
    s = np.zeros((b, c, h, w), dtype=np.float32)
    for di in range(3):
        for dj in range(3):
            s += xp[:, :, di:di + h, dj:dj + w]
    return s / 9.0


def _blur_matrix():
    x = np.arange(KS, dtype=np.float64) - (KS - 1) / 2.0
    g = np.exp(-0.5 * (x / SIGMA) ** 2)
    g = (g / g.sum()).astype(np.float32)
    p = KS // 2
    M = np.zeros((INPUT_SIZE, INPUT_SIZE), dtype=np.float32)
    for o in range(INPUT_SIZE):
        for t in range(KS):
            idx = o + t - p
            if idx < 0:
                idx = -idx
            elif idx > INPUT_SIZE - 1:
                idx = 2 * (INPUT_SIZE - 1) - idx
            M[o, idx] += g[t]
    return M


def kernel(feat_layer2, feat_layer3, memory_bank):
    from concourse.bass_utils import run_bass_kernel_spmd
    global LAST_RESULTS, LAST_IN_MAPS

    feat_layer2 = np.asarray(feat_layer2, dtype=np.float32)
    feat_layer3 = np.asarray(feat_layer3, dtype=np.float32)
    memory_bank = np.asarray(memory_bank, dtype=np.float32)

    # ---- host: build the embedding ------------------------------------
    f2 = _avg_pool3(feat_layer2)                       # [16,512,28,28]
    f3 = _avg_pool3(feat_layer3)                       # [16,1024,14,14]
    f3u = f3.repeat(2, axis=2).repeat(2, axis=3)       # [16,1024,28,28]
    emb = np.concatenate([f2, f3u], axis=1)            # [16,1536,28,28]
    emb = np.ascontiguousarray(emb.transpose(0, 2, 3, 1)).reshape(NPATCH, D)

    dev_dt = ml_dtypes.float8_e4m3fn if USE_FP8 else ml_dtypes.bfloat16
    embT = np.ascontiguousarray(emb.T)                 # [1536, 12544]
    embT_bf = embT.astype(dev_dt).reshape(KCH, 128, NPATCH)

    x2 = np.einsum("nd,nd->n", emb, emb).astype(np.float32)       # [12544]
    m2 = np.einsum("md,md->m", memory_bank, memory_bank).astype(np.float32)

    in_maps = []
    for c in range(NCORES):
        shard = memory_bank[c * MS:(c + 1) * MS]
        bankT_bf = np.ascontiguousarray(shard.T).astype(
            dev_dt).reshape(KCH, 128, MS)
        in_maps.append({
            "embT": embT_bf,
            "bankT": bankT_bf,
            "m2h": (0.5 * m2[c * MS:(c + 1) * MS]).astype(np.float32),
        })

    # ---- device: max_j (emb.bank_j - 0.5*m2_j) per patch per shard ----
    LAST_IN_MAPS = in_maps
    nc = _get_nc()
    LAST_RESULTS = run_bass_kernel_spmd(nc, in_maps, list(range(NCORES)))
    qs = np.stack([LAST_RESULTS.results[c]["qmax"].T.reshape(-1)
                   for c in range(NCORES)])            # [8, 12544]
    qglob = qs.max(axis=0)

    d2 = x2 - 2.0 * qglob
    s0 = np.sqrt(np.maximum(d2, 1e-12)).astype(np.float32)        # [12544]

    # ---- anomaly map: upsample x8 + separable gaussian blur -----------
    amap = s0.reshape(B, H2, W2).repeat(8, axis=1).repeat(8, axis=2)
    Mb = _blur_matrix()
    amap = np.matmul(np.matmul(Mb[None], amap), Mb.T[None])
    amap = amap.reshape(B, 1, INPUT_SIZE, INPUT_SIZE).astype(np.float32)

    # ---- anomaly score: exact top-9 for the argmax patch --------------
    ncand = 32
    cand = np.argpartition(-s0, ncand)[:ncand]
    ecand = emb[cand]                                   # [C, D]
    d2c = x2[cand][:, None] + m2[None, :] - 2.0 * (ecand @ memory_bank.T)
    dc = np.sqrt(np.maximum(d2c, 1e-12)).astype(np.float32)   # [C, 16384]
    s0c = dc.min(axis=1)
    ib = int(np.argmax(s0c))
    smax = np.float32(s0c[ib])
    drow = dc[ib]
    conf = np.sort(np.partition(drow, NUM_NEIGHBORS - 1)[:NUM_NEIGHBORS])
    econf = np.exp(conf)
    weights = np.float32(1.0 - econf.max() / econf.sum())
    anomaly_score = np.float32(weights * smax)

    return amap, anomaly_score
